# revision 12
# baseline (speedup 1.0000x reference)
"""Trainium2 Bass kernel for CombinedVectorField (CFG vector field + exact
Jacobian-trace divergence).

Math: with u = tanh(x@W1x + h@W1h + b1'), b1' = b1 + t*W1[256],
  v(x,h)  = u @ W2 + b2
  div(x,h)= sum_k (1-u_k^2) c_k = d0 - (u*u) @ c,   c_k = sum_i W1x[i,k] W2[k,i]
Output = concat[(1-gs)*v_null + gs*v_h, (1-gs)*div_null + gs*div_h].

Sharding: pure data parallel — each of the 8 cores takes 512 batch rows
(both guidance branches), weights replicated. All tensors are kept
feature-major (transposed) on device so every matmul contracts over the
partition dim; host does the transposes/reshapes only.
"""
import sys

sys.path.insert(0, "/opt/trn_rl_repo")

import ml_dtypes
import numpy as np

import concourse.bass as bass
import concourse.tile as tile
from concourse import bacc, mybir
from concourse.bass_utils import run_bass_kernel_spmd

F32 = mybir.dt.float32
BF16 = mybir.dt.bfloat16
AF = mybir.ActivationFunctionType
ALU = mybir.AluOpType

N_CORES = 8
B = 4096
DIM_X = 128
DIM_H = 128
HIDDEN = 512
R = B // N_CORES          # rows per core
NCH = HIDDEN // 128       # hidden chunks
W2W = NCH * DIM_X + NCH   # w2 chunks + cmat columns

_NC_CACHE = None


def _build():
    nc = bacc.Bacc("TRN2", target_bir_lowering=False, debug=False,
                   enable_asserts=False, monotonic_sem_count=0)

    # two merged bf16 input blobs, one per HWDGE ring:
    #   inA = [xT | w1x | w1h]  (gates the first matmuls)
    #   inB = [hT | hnT | w2r | cmat]
    inA = nc.dram_tensor("inA", [128, R + 2 * HIDDEN], BF16, kind="ExternalInput")
    inB = nc.dram_tensor("inB", [128, 2 * R + W2W], BF16, kind="ExternalInput")
    # aux cols: 0-3 b1' chunks, 4 b2, 5 gs, 6 1-gs, 7 -(1-gs), 8 d0, 9 -gs
    aux = nc.dram_tensor("aux", [128, 10], F32, kind="ExternalInput")

    VO = nc.dram_tensor("VO", [DIM_X, R], F32, kind="ExternalOutput")
    DO = nc.dram_tensor("DO", [1, R], F32, kind="ExternalOutput")

    with tile.TileContext(nc) as tc:
        with tc.tile_pool(name="cst", bufs=1) as cst, \
             tc.tile_pool(name="act", bufs=3) as actp, \
             tc.tile_pool(name="out", bufs=1) as outp, \
             tc.tile_pool(name="psa", bufs=4, space="PSUM") as psa, \
             tc.tile_pool(name="psv", bufs=1, space="PSUM") as psv:
            # PE prewarm: dummy f32 matmuls on a zeroed tile keep the PE-HAM
            # activity window busy during the input DMAs, so real matmuls run
            # at 2.4 GHz instead of 1.2 GHz.
            wrm = cst.tile([128, 256], F32)
            nc.gpsimd.memset(wrm[:], 0.0)
            pwarm = psa.tile([128, R], F32, tag="a")
            for _ in range(6):
                nc.tensor.matmul(pwarm[:, 0:256], wrm[:, 0:128], wrm[:],
                                 start=True, stop=True, skip_group_check=True)

            at = cst.tile([128, R + 2 * HIDDEN], BF16)
            nc.sync.dma_start(out=at[:], in_=inA[:])
            bt = cst.tile([128, 2 * R + W2W], BF16)
            nc.scalar.dma_start(out=bt[:], in_=inB[:])
            auxt = cst.tile([128, 10], F32)
            nc.gpsimd.dma_start(out=auxt[:], in_=aux[:])

            xt = at[:, 0:R]
            w1x = at[:, R:R + HIDDEN]
            w1h = at[:, R + HIDDEN:R + 2 * HIDDEN]
            hst = bt[:, 0:2 * R]
            w2t = bt[:, 2 * R:2 * R + NCH * DIM_X]
            cmt = bt[:, 2 * R + NCH * DIM_X:]

            pv = psv.tile([128, 2 * R], F32)       # v accum: [h | null]
            pd = psv.tile([1, 2 * R], F32)         # sum c*u^2: [h | null]

            # per-(chunk, branch) pieces: finer ACT/PSUM granularity keeps the
            # PE from stalling at chunk boundaries (4 rotating 1-bank a-tiles)
            for c in range(NCH):
                cs = bass.ts(c, 128)
                first, last = c == 0, c == NCH - 1
                for br in range(2):
                    bs = bass.ts(br, R)            # branch slice in hst/pv/pd
                    a = psa.tile([128, R], F32, tag="a")
                    nc.tensor.matmul(a[:], w1x[:, cs], xt[:], start=True, stop=False)
                    nc.tensor.matmul(a[:], w1h[:, cs], hst[:, bs], start=False, stop=True)

                    u = actp.tile([128, R], BF16, tag="u")
                    nc.scalar.activation(u[:], a[:], AF.Tanh, bias=auxt[:, c:c + 1], scale=1.0)
                    u2 = actp.tile([128, R], BF16, tag="u2")
                    nc.vector.tensor_tensor(u2[:], u[:], u[:], op=ALU.mult)

                    nc.tensor.matmul(pv[:, bs], w2t[:, cs], u[:], start=first, stop=last)
                    nc.tensor.matmul(pd[0:1, bs], cmt[:, c:c + 1], u2[:], start=first, stop=last)

            # v = gs*v_h + ((1-gs)*v_null + b2); t2 on ACT (idle), rest on DVE
            t2 = outp.tile([128, R], F32)
            nc.scalar.activation(t2[:], pv[:, R:2 * R], AF.Identity,
                                 bias=auxt[:, 4:5], scale=auxt[:, 6:7])
            vout = outp.tile([128, R], F32)
            nc.vector.scalar_tensor_tensor(vout[:], pv[:, 0:R], auxt[:, 5:6], t2[:],
                                           op0=ALU.mult, op1=ALU.add)
            # div = d0 - gs*s_h - (1-gs)*s_n; dt2 on ACT in parallel with vout
            dt2 = outp.tile([1, R], F32)
            nc.scalar.activation(dt2[:], pd[0:1, R:2 * R], AF.Identity,
                                 bias=auxt[0:1, 8:9], scale=auxt[0:1, 7:8])
            dout = outp.tile([1, R], F32)
            nc.vector.scalar_tensor_tensor(dout[:], pd[0:1, 0:R], auxt[0:1, 9:10], dt2[:],
                                           op0=ALU.mult, op1=ALU.add)

            nc.sync.dma_start(out=VO[:], in_=vout[:])
            nc.scalar.dma_start(out=DO[:], in_=dout[:])
    nc.compile()
    return nc


def _get_nc():
    global _NC_CACHE
    if _NC_CACHE is None:
        _NC_CACHE = _build()
    return _NC_CACHE


def _prep_in_maps(state, h, h_null, t, guidance_scale, W1, b1, W2, b2):
    f32 = np.float32
    bf = ml_dtypes.bfloat16
    xTf = state[:, :DIM_X].T.astype(bf)                            # (128, B)
    hTf = h.T.astype(bf)
    hnTf = h_null.T.astype(bf)
    w1f = np.concatenate([W1[:DIM_X], W1[DIM_X:DIM_X + DIM_H]], axis=1).astype(bf)
    b1p = (b1.astype(f32) + t.astype(f32)[0] * W1[DIM_X + DIM_H].astype(f32))
    w2r = W2.astype(f32).reshape(NCH, 128, DIM_X).transpose(1, 0, 2).reshape(128, NCH * DIM_X)
    cvec = (W1[:DIM_X].astype(np.float64) * W2.astype(np.float64).T).sum(0)  # (512,)
    d0 = cvec.sum()
    cmatf = cvec.reshape(NCH, 128).T.astype(f32)                   # (128, NCH)
    w2cf = np.concatenate([w2r, cmatf], axis=1).astype(bf)
    gs = float(guidance_scale.astype(f32)[0])

    auxf = np.zeros((128, 10), f32)
    auxf[:, 0:4] = b1p.reshape(NCH, 128).T
    auxf[:, 4] = b2.astype(f32)
    auxf[:, 5] = gs
    auxf[:, 6] = 1.0 - gs
    auxf[:, 7] = -(1.0 - gs)
    auxf[:, 8] = d0
    auxf[:, 9] = -gs

    in_maps = []
    for i in range(N_CORES):
        sl = slice(i * R, (i + 1) * R)
        in_maps.append({
            "inA": np.ascontiguousarray(
                np.concatenate([xTf[:, sl], w1f], axis=1)),
            "inB": np.ascontiguousarray(
                np.concatenate([hTf[:, sl], hnTf[:, sl], w2cf], axis=1)),
            "aux": auxf,
        })
    return in_maps


def kernel(state, h, h_null, t, guidance_scale, W1, b1, W2, b2, _trace=False):
    nc = _get_nc()
    in_maps = _prep_in_maps(state, h, h_null, t, guidance_scale, W1, b1, W2, b2)
    res = run_bass_kernel_spmd(nc, in_maps, list(range(N_CORES)), trace=_trace)
    out = np.empty((B, DIM_X + 1), np.float32)
    for i in range(N_CORES):
        sl = slice(i * R, (i + 1) * R)
        out[sl, :DIM_X] = res.results[i]["VO"].T
        out[sl, DIM_X] = res.results[i]["DO"][0]
    if _trace:
        return out, res
    return out
